# revision 25
# baseline (speedup 1.0000x reference)
"""Trainium2 Bass kernel for coverage-attention (Bahdanau + coverage).

Reference computation (fp32):
    enc   = encoder_outputs.transpose(1,0,2)            # [B,T,H]
    x     = concat([hidden_rep, enc, cov*W_cov], -1)    # [B,T,3H]
    energy= relu(x @ W_attn.T + b_attn)                 # [B,T,H]
    scores= energy @ v                                  # [B,T]
    attn  = softmax(scores, axis=1)
    out   = (attn[:,None,:], coverage + attn)

Decomposition used here (W_attn = [W1 | W2 | W3], each [H,H]):
    z[b,t,h] = (enc @ W2.T)[b,t,h] + a[b,h] + cov[b,t]*u[h]
      a = hidden[0] @ W1.T + b_attn      (tiny, host-precomputed)
      u = W3 @ W_cov[:,0]                (tiny, host-precomputed)
    scores[b,t] = sum_h v[h]*relu(z[b,t,h])

|v[h]| is folded into the h-columns of W2.T / u / a on the host
(relu(|v|*z) == |v|*relu(z)), and the h axis is permuted so all v>=0
columns come first.  Then scores = S_pos - S_neg over the two column
slices.

Per core (batch-parallel over 8 cores, 4 batches each), per [128t,512h]
tile:
  - PE: K=2 matmul ([cov;1].T @ [u;a_b]) seeds PSUM with the coverage
    rank-1 term + bias, then 4 bf16 matmuls [128k,128t].T @ [128k,512h]
    accumulate enc @ W2.T.  100%-utilization MACs; this is the
    bottleneck engine (~68us/core).
  - DVE: one tensor_scalar per sign-slice does relu+sign+reduce in a
    single op: out=(pz max 0)*(+-1), accum_out=sum -> score column.
  - softmax per batch: PE transpose [128,16]->[16,128], gpsimd
    partition reductions, ACT exp, DVE reciprocal/scale.
"""

import os
import sys

import numpy as np

for _p in ("/opt/trn_rl_repo", "/root/.axon_site/_ro/trn_rl_repo"):
    if os.path.isdir(_p) and _p not in sys.path:
        sys.path.insert(0, _p)
        break

import ml_dtypes  # noqa: E402

H = 512
B = 32
T = 2048
N_CORES = 8
BPC = B // N_CORES          # batches per core
TC = T // 128               # 16 score columns (t-tiles) per batch
TCG = 4                     # t-tile groups of 512 t each
KC = 4                      # k chunks of 128

_PROGRAM_CACHE: dict = {}


def _build_program(p_pos: int, reps: int = 1):
    """Build + compile the single-core Bass/Tile program (SPMD across 8).

    reps>1 repeats the whole computation back-to-back inside one NEFF
    (idempotent), for wall-clock benchmarking that cancels host overhead.
    """
    from contextlib import ExitStack

    import concourse.tile as tile
    from concourse import bacc, mybir

    f32 = mybir.dt.float32
    bf16 = mybir.dt.bfloat16
    Alu = mybir.AluOpType
    Act = mybir.ActivationFunctionType

    nc = bacc.Bacc(
        "TRN2",
        target_bir_lowering=False,
        debug=False,
        enable_asserts=False,
        num_devices=N_CORES,
    )

    enc_d = nc.dram_tensor("enc_in", [BPC, H, T], bf16, kind="ExternalInput").ap()
    w2t_d = nc.dram_tensor("w2t_in", [H, H], bf16, kind="ExternalInput").ap()
    covx_d = nc.dram_tensor("covx_in", [2, BPC, T], bf16, kind="ExternalInput").ap()
    rhx_d = nc.dram_tensor("rhx_in", [2, BPC, H], bf16, kind="ExternalInput").ap()
    covt_d = nc.dram_tensor("covt_in", [BPC, TC, 128], f32, kind="ExternalInput").ap()
    idn_d = nc.dram_tensor("iden_in", [128, 128], f32, kind="ExternalInput").ap()
    # fused output: [..., 0:128] = attn, [..., 128:256] = coverage_new
    out_d = nc.dram_tensor("out2_out", [BPC, TC, 256], f32, kind="ExternalOutput").ap()

    with tile.TileContext(nc) as tc, ExitStack() as ctx:
        singles = ctx.enter_context(tc.tile_pool(name="singles", bufs=1))
        encp = ctx.enter_context(tc.tile_pool(name="encp", bufs=4))
        encpb = ctx.enter_context(tc.tile_pool(name="encpb", bufs=8))
        scrapp = ctx.enter_context(tc.tile_pool(name="scrapp", bufs=3))
        scorep = ctx.enter_context(tc.tile_pool(name="scorep", bufs=2))
        smallp = ctx.enter_context(tc.tile_pool(name="smallp", bufs=2))
        outp = ctx.enter_context(tc.tile_pool(name="outp", bufs=2))
        psum = ctx.enter_context(tc.tile_pool(name="psum", bufs=4, space="PSUM"))
        psum_t = ctx.enter_context(tc.tile_pool(name="psum_t", bufs=2, space="PSUM"))
        psum_s = ctx.enter_context(tc.tile_pool(name="psum_s", bufs=1, space="PSUM"))

        # --- constants; interleave first enc group so PE starts ASAP ---
        covx_sb = singles.tile([2, BPC, T], bf16)
        nc.sync.dma_start(out=covx_sb[:], in_=covx_d[:])
        rhx_sb = singles.tile([2, BPC, H], bf16)
        nc.sync.dma_start(out=rhx_sb[:], in_=rhx_d[:])
        w2t_sb = singles.tile([128, KC, H], bf16)
        first_group = []
        for kc in range(KC):
            nc.sync.dma_start(out=w2t_sb[:, kc, :], in_=w2t_d[kc * 128:(kc + 1) * 128, :])
            et = encp.tile([128, 512], bf16, tag="enc_t0")
            nc.sync.dma_start(out=et[:], in_=enc_d[0, kc * 128:(kc + 1) * 128, 0:512])
            first_group.append(et)
        # rest of b0 rows in one medium DMA per k-chunk
        b0_rest = []
        for kc in range(KC):
            et = encp.tile([128, 3 * 512], bf16, tag="enc_t1")
            nc.sync.dma_start(out=et[:], in_=enc_d[0, kc * 128:(kc + 1) * 128, 512:T])
            b0_rest.append(et)
        # cold constants (not needed until the first epilogue) on another queue
        covt_sb = singles.tile([TC, BPC, 128], f32)
        nc.scalar.dma_start(
            out=covt_sb[:], in_=covt_d.rearrange("b q j -> q b j")
        )
        idn_sb = singles.tile([128, 128], f32)
        nc.scalar.dma_start(out=idn_sb[:], in_=idn_d[:])
        ones_c = singles.tile([TC, 1], f32)
        nc.vector.memset(ones_c[:], 1.0)
        ones_r = singles.tile([1, TC], f32)
        nc.vector.memset(ones_r[:], 1.0)

        big_tiles: dict = {}
        for rep in range(reps):
          for b in range(BPC):
            sp = scorep.tile([128, TC], f32, tag="sp")
            sm = scorep.tile([128, TC], f32, tag="sm")
            if p_pos == 0:
                nc.vector.memset(sp[:], 0.0)
            if p_pos == H:
                nc.vector.memset(sm[:], 0.0)
            # prefetch the whole next batch as 4 big DMAs (fixed per-DMA
            # HWDGE descriptor-gen cost dominates; batch to amortize)
            if b + 1 < BPC:
                nxt = []
                for kc in range(KC):
                    et = encpb.tile([128, T], bf16, tag="enc_big")
                    nc.sync.dma_start(
                        out=et[:], in_=enc_d[b + 1, kc * 128:(kc + 1) * 128, :]
                    )
                    nxt.append(et)
                big_tiles[b + 1] = nxt
            for tcg in range(TCG):
                for j in range(4):
                    tci = tcg * 4 + j
                    pz = psum.tile([128, H], f32, tag="pz")
                    # K=2 rank-1 seed: [cov;1].T @ [u_s; a_b]  ->  cov*u + cb
                    nc.tensor.matmul(
                        pz[:],
                        lhsT=covx_sb[:, b, tci * 128:(tci + 1) * 128],
                        rhs=rhx_sb[:, b, :],
                        start=True,
                        stop=False,
                    )
                    for kc in range(KC):
                        if b == 0 and tcg == 0:
                            lhsT = first_group[kc][:, j * 128:(j + 1) * 128]
                        elif b == 0:
                            lhsT = b0_rest[kc][
                                :, (tcg - 1) * 512 + j * 128:(tcg - 1) * 512 + (j + 1) * 128
                            ]
                        else:
                            lhsT = big_tiles[b][kc][
                                :, tcg * 512 + j * 128:tcg * 512 + (j + 1) * 128
                            ]
                        nc.tensor.matmul(
                            pz[:],
                            lhsT=lhsT,
                            rhs=w2t_sb[:, kc, :],
                            start=False,
                            stop=(kc == KC - 1),
                        )
                    # relu + reduce in one DVE op per sign-slice:
                    # out = max(pz, 0); accum_out = reduce(out, op1=add)
                    if p_pos > 0:
                        scr = scrapp.tile([128, H], f32, tag="scr")
                        nc.vector.tensor_scalar(
                            out=scr[:, 0:p_pos],
                            in0=pz[:, 0:p_pos],
                            scalar1=0.0,
                            scalar2=None,
                            op0=Alu.max,
                            op1=Alu.add,
                            accum_out=sp[:, tci:tci + 1],
                        )
                    if p_pos < H:
                        scr2 = scrapp.tile([128, H], f32, tag="scr2")
                        nc.vector.tensor_scalar(
                            out=scr2[:, p_pos:H],
                            in0=pz[:, p_pos:H],
                            scalar1=0.0,
                            scalar2=None,
                            op0=Alu.max,
                            op1=Alu.add,
                            accum_out=sm[:, tci:tci + 1],
                        )

            # ---- per-batch epilogue: softmax over all 2048 t ----
            s_sb = smallp.tile([128, TC], f32, tag="s_sb")
            nc.vector.tensor_sub(s_sb[:], sp[:], sm[:])
            ps_t = psum_t.tile([TC, 128], f32, tag="ps_t")
            nc.tensor.transpose(ps_t[:], s_sb[:], idn_sb[:])
            # scores are O(3), so exp needs no max-subtraction (softmax is
            # shift-invariant; reference only subtracts max for range safety)
            expT = smallp.tile([TC, 128], f32, tag="expT")
            rsum = smallp.tile([TC, 1], f32, tag="rsum")
            nc.scalar.activation(
                out=expT[:], in_=ps_t[:], func=Act.Exp, accum_out=rsum[:]
            )
            # partition-sum and broadcast via tiny PE matmuls (gpsimd ops
            # force multi-ms ucode library reloads -- never use them here)
            zt_ps = psum_s.tile([1, 1], f32, tag="zt_ps")
            nc.tensor.matmul(zt_ps[:], lhsT=rsum[:], rhs=ones_c[:])
            zt = smallp.tile([1, 1], f32, tag="zt")
            nc.vector.tensor_copy(zt[:], zt_ps[:])
            rz = smallp.tile([1, 1], f32, tag="rz")
            nc.vector.reciprocal(rz[:], zt[:])
            rzb_ps = psum_s.tile([TC, 1], f32, tag="rzb_ps")
            nc.tensor.matmul(rzb_ps[:], lhsT=ones_r[:], rhs=rz[:])
            rzb = smallp.tile([TC, 1], f32, tag="rzb")
            nc.vector.tensor_copy(rzb[:], rzb_ps[:])
            o = outp.tile([TC, 256], f32, tag="o")
            nc.vector.tensor_scalar_mul(o[:, 0:128], expT[:], rzb[:])
            nc.vector.tensor_add(o[:, 128:256], o[:, 0:128], covt_sb[:, b, :])
            nc.sync.dma_start(out=out_d[b], in_=o[:])

    nc.compile()
    return nc


def _get_program(p_pos: int, reps: int = 1):
    key = (p_pos, reps)
    if key not in _PROGRAM_CACHE:
        _PROGRAM_CACHE[key] = _build_program(p_pos, reps)
    return _PROGRAM_CACHE[key]


def _prepare(hidden, encoder_outputs, coverage, W_attn, b_attn, v, W_cov):
    """Host-side sharding + weight folding. Returns (p_pos, in_maps)."""
    hidden = np.asarray(hidden, dtype=np.float32)
    encoder_outputs = np.asarray(encoder_outputs, dtype=np.float32)
    coverage = np.asarray(coverage, dtype=np.float32)
    W_attn = np.asarray(W_attn, dtype=np.float32)
    b_attn = np.asarray(b_attn, dtype=np.float32)
    v = np.asarray(v, dtype=np.float32)
    W_cov = np.asarray(W_cov, dtype=np.float32)

    W1 = W_attn[:, :H].astype(np.float64)
    W2 = W_attn[:, H:2 * H].astype(np.float64)
    W3 = W_attn[:, 2 * H:].astype(np.float64)
    u = W3 @ W_cov[:, 0].astype(np.float64)                      # [H]
    a = hidden[0].astype(np.float64) @ W1.T + b_attn.astype(np.float64)  # [B,H]

    order = np.argsort(v < 0, kind="stable")                     # v>=0 first
    p_pos = int((v >= 0).sum())
    vabs = np.abs(v[order].astype(np.float64))

    w2t_s = (W2[order, :].T * vabs[None, :])                     # [k, h']
    w2t_bf = w2t_s.astype(np.float32).astype(ml_dtypes.bfloat16)
    u_s = (u[order] * vabs).astype(np.float32)                   # [H]
    cb_s = (a[:, order] * vabs[None, :]).astype(np.float32)      # [B, H]

    ident = np.eye(128, dtype=np.float32)

    in_maps = []
    for c in range(N_CORES):
        bs = slice(c * BPC, (c + 1) * BPC)
        e = encoder_outputs[:, bs, :]                            # [T, BPC, H]
        enc_bf = np.ascontiguousarray(e.transpose(1, 2, 0)).astype(
            ml_dtypes.bfloat16
        )                                                        # [BPC, H, T]
        cov_c = coverage[bs]                                     # [BPC, T]
        covt = np.ascontiguousarray(cov_c.reshape(BPC, TC, 128))
        covx = np.empty((2, BPC, T), dtype=ml_dtypes.bfloat16)
        covx[0] = cov_c.astype(ml_dtypes.bfloat16)
        covx[1] = np.float32(1.0)
        rhx = np.empty((2, BPC, H), dtype=ml_dtypes.bfloat16)
        rhx[0] = u_s[None, :].astype(ml_dtypes.bfloat16)
        rhx[1] = cb_s[bs].astype(ml_dtypes.bfloat16)
        in_maps.append(
            {
                "enc_in": enc_bf,
                "w2t_in": w2t_bf,
                "covx_in": covx,
                "rhx_in": rhx,
                "covt_in": covt,
                "iden_in": ident,
            }
        )
    return p_pos, in_maps


def _run(inputs: dict, trace: bool = False, reps: int = 1):
    """Run on 8 NeuronCores. Returns ((attn, covnew), BassKernelResults)."""
    from concourse import bass_utils

    p_pos, in_maps = _prepare(**inputs)
    nc = _get_program(p_pos, reps)
    res = bass_utils.run_bass_kernel_spmd(
        nc, in_maps, core_ids=list(range(N_CORES)), trace=trace
    )
    outs = np.concatenate(
        [res.results[c]["out2_out"] for c in range(N_CORES)], axis=0
    ).astype(np.float32)                                        # [B, TC, 256]
    attn = np.ascontiguousarray(outs[:, :, 0:128]).reshape(B, T)
    covn = np.ascontiguousarray(outs[:, :, 128:256]).reshape(B, T)
    return (attn[:, None, :], covn), res


def kernel(hidden, encoder_outputs, coverage, W_attn, b_attn, v, W_cov):
    out, _ = _run(
        dict(
            hidden=hidden,
            encoder_outputs=encoder_outputs,
            coverage=coverage,
            W_attn=W_attn,
            b_attn=b_attn,
            v=v,
            W_cov=W_cov,
        )
    )
    return out


# revision 31
# speedup vs baseline: 1.6105x; 1.6105x over previous
"""Trainium2 Bass kernel for coverage-attention (Bahdanau + coverage).

Reference computation (fp32):
    enc   = encoder_outputs.transpose(1,0,2)            # [B,T,H]
    x     = concat([hidden_rep, enc, cov*W_cov], -1)    # [B,T,3H]
    energy= relu(x @ W_attn.T + b_attn)                 # [B,T,H]
    scores= energy @ v                                  # [B,T]
    attn  = softmax(scores, axis=1)
    out   = (attn[:,None,:], coverage + attn)

Decomposition used here (W_attn = [W1 | W2 | W3], each [H,H]):
    z[b,t,h] = (enc @ W2.T)[b,t,h] + a[b,h] + cov[b,t]*u[h]
      a = hidden[0] @ W1.T + b_attn      (tiny, host-precomputed)
      u = W3 @ W_cov[:,0]                (tiny, host-precomputed)
    scores[b,t] = sum_h v[h]*relu(z[b,t,h])

|v[h]| is folded into the h-columns of W2.T / u / a on the host
(relu(|v|*z) == |v|*relu(z)), and the h axis is permuted so all v>=0
columns come first.  Then scores = S_pos - S_neg over the two column
slices.

Per core (batch-parallel over 8 cores, 4 batches each), per [128t,512h]
tile:
  - PE: K=2 matmul ([cov;1].T @ [u;a_b]) seeds PSUM with the coverage
    rank-1 term + bias, then 4 bf16 matmuls [128k,128t].T @ [128k,512h]
    accumulate enc @ W2.T.  100%-utilization MACs; this is the
    bottleneck engine (~68us/core).
  - DVE: one tensor_scalar per sign-slice does relu+sign+reduce in a
    single op: out=(pz max 0)*(+-1), accum_out=sum -> score column.
  - softmax per batch: PE transpose [128,16]->[16,128], gpsimd
    partition reductions, ACT exp, DVE reciprocal/scale.
"""

import os
import sys

import numpy as np

for _p in ("/opt/trn_rl_repo", "/root/.axon_site/_ro/trn_rl_repo"):
    if os.path.isdir(_p) and _p not in sys.path:
        sys.path.insert(0, _p)
        break

import ml_dtypes  # noqa: E402

H = 512
B = 32
T = 2048
N_CORES = 8
BPC = B // N_CORES          # batches per core
TC = T // 128               # 16 score columns (t-tiles) per batch
TCG = 4                     # t-tile groups of 512 t each
KC = 4                      # k chunks of 128

_PROGRAM_CACHE: dict = {}


def _build_program(p_pos: int, reps: int = 1, mode: str = "full"):
    """Build + compile the single-core Bass/Tile program (SPMD across 8).

    reps>1 repeats the whole computation back-to-back inside one NEFF
    (idempotent), for wall-clock benchmarking that cancels host overhead.
    mode: "full" | "dma" (loads only) | "mm" (loads+matmuls) for bisection.
    """
    from contextlib import ExitStack

    import concourse.tile as tile
    from concourse import bacc, mybir

    f32 = mybir.dt.float32
    bf16 = mybir.dt.bfloat16
    Alu = mybir.AluOpType
    Act = mybir.ActivationFunctionType

    nc = bacc.Bacc(
        "TRN2",
        target_bir_lowering=False,
        debug=False,
        enable_asserts=False,
        num_devices=N_CORES,
    )

    enc_d = nc.dram_tensor("enc_in", [BPC, H, T], bf16, kind="ExternalInput").ap()
    w2t_d = nc.dram_tensor("w2t_in", [H, H], bf16, kind="ExternalInput").ap()
    covx_d = nc.dram_tensor("covx_in", [2, BPC, T], bf16, kind="ExternalInput").ap()
    rhx_d = nc.dram_tensor("rhx_in", [2, BPC, H], bf16, kind="ExternalInput").ap()
    covt_d = nc.dram_tensor("covt_in", [BPC, TC, 128], f32, kind="ExternalInput").ap()
    idn_d = nc.dram_tensor("iden_in", [128, 128], f32, kind="ExternalInput").ap()
    # fused output: [..., 0:128] = attn, [..., 128:256] = coverage_new
    out_d = nc.dram_tensor("out2_out", [BPC, TC, 256], f32, kind="ExternalOutput").ap()

    with tile.TileContext(nc) as tc, ExitStack() as ctx:
        singles = ctx.enter_context(tc.tile_pool(name="singles", bufs=1))
        encp = ctx.enter_context(tc.tile_pool(name="encp", bufs=4))
        encpb = ctx.enter_context(tc.tile_pool(name="encpb", bufs=8))
        scrapp = ctx.enter_context(tc.tile_pool(name="scrapp", bufs=3))
        scorep = ctx.enter_context(tc.tile_pool(name="scorep", bufs=2))
        smallp = ctx.enter_context(tc.tile_pool(name="smallp", bufs=2))
        outp = ctx.enter_context(tc.tile_pool(name="outp", bufs=2))
        psum = ctx.enter_context(tc.tile_pool(name="psum", bufs=4, space="PSUM"))
        psum_t = ctx.enter_context(tc.tile_pool(name="psum_t", bufs=2, space="PSUM"))
        psum_s = ctx.enter_context(tc.tile_pool(name="psum_s", bufs=1, space="PSUM"))

        # --- constants; interleave first enc group so PE starts ASAP ---
        covx_sb = singles.tile([2, BPC, T], bf16)
        nc.sync.dma_start(out=covx_sb[:], in_=covx_d[:])
        rhx_sb = singles.tile([2, BPC, H], bf16)
        nc.sync.dma_start(out=rhx_sb[:], in_=rhx_d[:])
        w2t_sb = singles.tile([128, KC, H], bf16)
        first_group = []
        for kc in range(KC):
            nc.sync.dma_start(out=w2t_sb[:, kc, :], in_=w2t_d[kc * 128:(kc + 1) * 128, :])
            et = encp.tile([128, 512], bf16, tag="enc_t0")
            nc.sync.dma_start(out=et[:], in_=enc_d[0, kc * 128:(kc + 1) * 128, 0:512])
            first_group.append(et)
        # rest of b0 rows in one medium DMA per k-chunk
        b0_rest = []
        for kc in range(KC):
            et = encp.tile([128, 3 * 512], bf16, tag="enc_t1")
            nc.sync.dma_start(out=et[:], in_=enc_d[0, kc * 128:(kc + 1) * 128, 512:T])
            b0_rest.append(et)
        # cold constants (not needed until the first epilogue) on another queue
        covt_sb = singles.tile([TC, BPC, 128], f32)
        nc.scalar.dma_start(
            out=covt_sb[:], in_=covt_d.rearrange("b q j -> q b j")
        )
        idn_sb = singles.tile([128, 128], f32)
        nc.scalar.dma_start(out=idn_sb[:], in_=idn_d[:])
        ones_c = singles.tile([TC, 1], f32)
        nc.vector.memset(ones_c[:], 1.0)
        ones_r = singles.tile([1, TC], f32)
        nc.vector.memset(ones_r[:], 1.0)

        big_tiles: dict = {}
        for rep in range(reps):
          for b in range(BPC):
            sp = scorep.tile([128, TC], f32, tag="sp")
            sm = scorep.tile([128, TC], f32, tag="sm")
            if p_pos == 0:
                nc.vector.memset(sp[:], 0.0)
            if p_pos == H:
                nc.vector.memset(sm[:], 0.0)
            # prefetch the whole next batch as 4 big DMAs (fixed per-DMA
            # HWDGE descriptor-gen cost dominates; batch to amortize)
            if b + 1 < BPC:
                nxt = []
                for kc in range(KC):
                    et = encpb.tile([128, T], bf16, tag="enc_big")
                    nc.sync.dma_start(
                        out=et[:], in_=enc_d[b + 1, kc * 128:(kc + 1) * 128, :]
                    )
                    nxt.append(et)
                big_tiles[b + 1] = nxt
            for tcg in range(TCG):
                for j in range(4):
                    if mode == "dma":
                        continue
                    tci = tcg * 4 + j
                    pz = psum.tile([128, H], f32, tag="pz")
                    # K=2 rank-1 seed: [cov;1].T @ [u_s; a_b]  ->  cov*u + cb
                    nc.tensor.matmul(
                        pz[:],
                        lhsT=covx_sb[:, b, tci * 128:(tci + 1) * 128],
                        rhs=rhx_sb[:, b, :],
                        start=True,
                        stop=False,
                    )
                    for kc in range(KC):
                        if b == 0 and tcg == 0:
                            lhsT = first_group[kc][:, j * 128:(j + 1) * 128]
                        elif b == 0:
                            lhsT = b0_rest[kc][
                                :, (tcg - 1) * 512 + j * 128:(tcg - 1) * 512 + (j + 1) * 128
                            ]
                        else:
                            lhsT = big_tiles[b][kc][
                                :, tcg * 512 + j * 128:tcg * 512 + (j + 1) * 128
                            ]
                        nc.tensor.matmul(
                            pz[:],
                            lhsT=lhsT,
                            rhs=w2t_sb[:, kc, :],
                            start=False,
                            stop=(kc == KC - 1),
                        )
                    if mode == "mm":
                        continue
                    # relu + reduce in one DVE op per sign-slice:
                    # out = max(pz, 0); accum_out = reduce(out, op1=add)
                    if p_pos > 0:
                        scr = scrapp.tile([128, H], f32, tag="scr")
                        nc.vector.tensor_scalar(
                            out=scr[:, 0:p_pos],
                            in0=pz[:, 0:p_pos],
                            scalar1=0.0,
                            scalar2=None,
                            op0=Alu.max,
                            op1=Alu.add,
                            accum_out=sp[:, tci:tci + 1],
                        )
                    if p_pos < H:
                        scr2 = scrapp.tile([128, H], f32, tag="scr2")
                        nc.vector.tensor_scalar(
                            out=scr2[:, p_pos:H],
                            in0=pz[:, p_pos:H],
                            scalar1=0.0,
                            scalar2=None,
                            op0=Alu.max,
                            op1=Alu.add,
                            accum_out=sm[:, tci:tci + 1],
                        )

            # ---- per-batch epilogue: softmax over all 2048 t ----
            if mode in ("dma", "mm"):
                o = outp.tile([TC, 256], f32, tag="o")
                nc.vector.memset(o[:], 0.0)
                nc.sync.dma_start(out=out_d[b], in_=o[:])
                continue
            s_sb = smallp.tile([128, TC], f32, tag="s_sb")
            nc.vector.tensor_sub(s_sb[:], sp[:], sm[:])
            ps_t = psum_t.tile([TC, 128], f32, tag="ps_t")
            nc.tensor.transpose(ps_t[:], s_sb[:], idn_sb[:])
            # scores are O(3), so exp needs no max-subtraction (softmax is
            # shift-invariant; reference only subtracts max for range safety)
            expT = smallp.tile([TC, 128], f32, tag="expT")
            rsum = smallp.tile([TC, 1], f32, tag="rsum")
            nc.scalar.activation(
                out=expT[:], in_=ps_t[:], func=Act.Exp, accum_out=rsum[:]
            )
            # partition-sum and broadcast via tiny PE matmuls (gpsimd ops
            # force multi-ms ucode library reloads -- never use them here)
            zt_ps = psum_s.tile([1, 1], f32, tag="zt_ps")
            nc.tensor.matmul(zt_ps[:], lhsT=rsum[:], rhs=ones_c[:])
            zt = smallp.tile([1, 1], f32, tag="zt")
            nc.vector.tensor_copy(zt[:], zt_ps[:])
            rz = smallp.tile([1, 1], f32, tag="rz")
            nc.vector.reciprocal(rz[:], zt[:])
            rzb_ps = psum_s.tile([TC, 1], f32, tag="rzb_ps")
            nc.tensor.matmul(rzb_ps[:], lhsT=ones_r[:], rhs=rz[:])
            rzb = smallp.tile([TC, 1], f32, tag="rzb")
            nc.vector.tensor_copy(rzb[:], rzb_ps[:])
            o = outp.tile([TC, 256], f32, tag="o")
            nc.vector.tensor_scalar_mul(o[:, 0:128], expT[:], rzb[:])
            nc.vector.tensor_add(o[:, 128:256], o[:, 0:128], covt_sb[:, b, :])
            nc.sync.dma_start(out=out_d[b], in_=o[:])

    nc.compile()
    return nc


def _get_program(p_pos: int, reps: int = 1, mode: str = "full"):
    key = (p_pos, reps, mode)
    if key not in _PROGRAM_CACHE:
        _PROGRAM_CACHE[key] = _build_program(p_pos, reps, mode)
    return _PROGRAM_CACHE[key]


def _prepare(hidden, encoder_outputs, coverage, W_attn, b_attn, v, W_cov):
    """Host-side sharding + weight folding. Returns (p_pos, in_maps)."""
    hidden = np.asarray(hidden, dtype=np.float32)
    encoder_outputs = np.asarray(encoder_outputs, dtype=np.float32)
    coverage = np.asarray(coverage, dtype=np.float32)
    W_attn = np.asarray(W_attn, dtype=np.float32)
    b_attn = np.asarray(b_attn, dtype=np.float32)
    v = np.asarray(v, dtype=np.float32)
    W_cov = np.asarray(W_cov, dtype=np.float32)

    W1 = W_attn[:, :H].astype(np.float64)
    W2 = W_attn[:, H:2 * H].astype(np.float64)
    W3 = W_attn[:, 2 * H:].astype(np.float64)
    u = W3 @ W_cov[:, 0].astype(np.float64)                      # [H]
    a = hidden[0].astype(np.float64) @ W1.T + b_attn.astype(np.float64)  # [B,H]

    order = np.argsort(v < 0, kind="stable")                     # v>=0 first
    p_pos = int((v >= 0).sum())
    vabs = np.abs(v[order].astype(np.float64))

    w2t_s = (W2[order, :].T * vabs[None, :])                     # [k, h']
    w2t_bf = w2t_s.astype(np.float32).astype(ml_dtypes.bfloat16)
    u_s = (u[order] * vabs).astype(np.float32)                   # [H]
    cb_s = (a[:, order] * vabs[None, :]).astype(np.float32)      # [B, H]

    ident = np.eye(128, dtype=np.float32)

    in_maps = []
    for c in range(N_CORES):
        bs = slice(c * BPC, (c + 1) * BPC)
        e = encoder_outputs[:, bs, :]                            # [T, BPC, H]
        enc_bf = np.ascontiguousarray(e.transpose(1, 2, 0)).astype(
            ml_dtypes.bfloat16
        )                                                        # [BPC, H, T]
        cov_c = coverage[bs]                                     # [BPC, T]
        covt = np.ascontiguousarray(cov_c.reshape(BPC, TC, 128))
        covx = np.empty((2, BPC, T), dtype=ml_dtypes.bfloat16)
        covx[0] = cov_c.astype(ml_dtypes.bfloat16)
        covx[1] = np.float32(1.0)
        rhx = np.empty((2, BPC, H), dtype=ml_dtypes.bfloat16)
        rhx[0] = u_s[None, :].astype(ml_dtypes.bfloat16)
        rhx[1] = cb_s[bs].astype(ml_dtypes.bfloat16)
        in_maps.append(
            {
                "enc_in": enc_bf,
                "w2t_in": w2t_bf,
                "covx_in": covx,
                "rhx_in": rhx,
                "covt_in": covt,
                "iden_in": ident,
            }
        )
    return p_pos, in_maps


def _run(inputs: dict, trace: bool = False, reps: int = 1, mode: str = "full"):
    """Run on 8 NeuronCores. Returns ((attn, covnew), BassKernelResults)."""
    from concourse import bass_utils

    p_pos, in_maps = _prepare(**inputs)
    nc = _get_program(p_pos, reps, mode)
    res = bass_utils.run_bass_kernel_spmd(
        nc, in_maps, core_ids=list(range(N_CORES)), trace=trace
    )
    outs = np.concatenate(
        [res.results[c]["out2_out"] for c in range(N_CORES)], axis=0
    ).astype(np.float32)                                        # [B, TC, 256]
    attn = np.ascontiguousarray(outs[:, :, 0:128]).reshape(B, T)
    covn = np.ascontiguousarray(outs[:, :, 128:256]).reshape(B, T)
    return (attn[:, None, :], covn), res


def kernel(hidden, encoder_outputs, coverage, W_attn, b_attn, v, W_cov):
    out, _ = _run(
        dict(
            hidden=hidden,
            encoder_outputs=encoder_outputs,
            coverage=coverage,
            W_attn=W_attn,
            b_attn=b_attn,
            v=v,
            W_cov=W_cov,
        )
    )
    return out


# revision 44
# speedup vs baseline: 207.3469x; 128.7488x over previous
"""Trainium2 Bass kernel for coverage-attention (Bahdanau + coverage).

Reference computation (fp32):
    enc   = encoder_outputs.transpose(1,0,2)            # [B,T,H]
    x     = concat([hidden_rep, enc, cov*W_cov], -1)    # [B,T,3H]
    energy= relu(x @ W_attn.T + b_attn)                 # [B,T,H]
    scores= energy @ v                                  # [B,T]
    attn  = softmax(scores, axis=1)
    out   = (attn[:,None,:], coverage + attn)

Decomposition used here (W_attn = [W1 | W2 | W3], each [H,H]):
    z[b,t,h] = (enc @ W2.T)[b,t,h] + a[b,h] + cov[b,t]*u[h]
      a = hidden[0] @ W1.T + b_attn      (tiny, host-precomputed)
      u = W3 @ W_cov[:,0]                (tiny, host-precomputed)
    scores[b,t] = sum_h v[h]*relu(z[b,t,h])

|v[h]| is folded into the h-columns of W2.T / u / a on the host
(relu(|v|*z) == |v|*relu(z)), and the h axis is permuted so all v>=0
columns come first.  Then scores = S_pos - S_neg over the two column
slices.

Per core (batch-parallel over 8 cores, 4 batches each), per [128t,512h]
tile:
  - PE: K=2 matmul ([cov;1].T @ [u;a_b]) seeds PSUM with the coverage
    rank-1 term + bias, then 4 bf16 matmuls [128k,128t].T @ [128k,512h]
    accumulate enc @ W2.T.  100%-utilization MACs; this is the
    bottleneck engine (~68us/core).
  - DVE: one tensor_scalar per sign-slice does relu+sign+reduce in a
    single op: out=(pz max 0)*(+-1), accum_out=sum -> score column.
  - softmax per batch: PE transpose [128,16]->[16,128], gpsimd
    partition reductions, ACT exp, DVE reciprocal/scale.
"""

import os
import sys

import numpy as np

for _p in ("/opt/trn_rl_repo", "/root/.axon_site/_ro/trn_rl_repo"):
    if os.path.isdir(_p) and _p not in sys.path:
        sys.path.insert(0, _p)
        break

import ml_dtypes  # noqa: E402

H = 512
B = 32
T = 2048
N_CORES = 8
BPC = B // N_CORES          # batches per core
TC = T // 128               # 16 score columns (t-tiles) per batch
TCG = 4                     # t-tile groups of 512 t each
KC = 4                      # k chunks of 128

_PROGRAM_CACHE: dict = {}


def _build_program(p_pos: int, reps: int = 1, mode: str = "full"):
    """Build + compile the single-core Bass/Tile program (SPMD across 8).

    reps>1 repeats the whole computation back-to-back inside one NEFF
    (idempotent), for wall-clock benchmarking that cancels host overhead.
    mode: "full" | "dma" (loads only) | "mm" (loads+matmuls) for bisection.
    """
    from contextlib import ExitStack

    import concourse.tile as tile
    from concourse import bacc, mybir

    f32 = mybir.dt.float32
    bf16 = mybir.dt.bfloat16
    Alu = mybir.AluOpType
    Act = mybir.ActivationFunctionType

    nc = bacc.Bacc(
        "TRN2",
        target_bir_lowering=False,
        debug=False,
        enable_asserts=False,
        num_devices=N_CORES,
    )

    enc_d = nc.dram_tensor("enc_in", [BPC, H, T], bf16, kind="ExternalInput").ap()
    w2t_d = nc.dram_tensor("w2t_in", [H, H], bf16, kind="ExternalInput").ap()
    covx_d = nc.dram_tensor("covx_in", [2, BPC, T], bf16, kind="ExternalInput").ap()
    rhx_d = nc.dram_tensor("rhx_in", [2, BPC, H], bf16, kind="ExternalInput").ap()
    covt_d = nc.dram_tensor("covt_in", [BPC, TC, 128], f32, kind="ExternalInput").ap()
    idn_d = nc.dram_tensor("iden_in", [128, 128], f32, kind="ExternalInput").ap()
    # fused output: [..., 0:128] = attn, [..., 128:256] = coverage_new
    out_d = nc.dram_tensor("out2_out", [BPC, TC, 256], f32, kind="ExternalOutput").ap()

    with tile.TileContext(nc) as tc, ExitStack() as ctx:
        singles = ctx.enter_context(tc.tile_pool(name="singles", bufs=1))
        encp = ctx.enter_context(tc.tile_pool(name="encp", bufs=4))
        encpb = ctx.enter_context(tc.tile_pool(name="encpb", bufs=8))
        scrapp = ctx.enter_context(tc.tile_pool(name="scrapp", bufs=3))
        scorep = ctx.enter_context(tc.tile_pool(name="scorep", bufs=2))
        smallp = ctx.enter_context(tc.tile_pool(name="smallp", bufs=2))
        outp = ctx.enter_context(tc.tile_pool(name="outp", bufs=2))
        psum = ctx.enter_context(tc.tile_pool(name="psum", bufs=4, space="PSUM"))
        psum_t = ctx.enter_context(tc.tile_pool(name="psum_t", bufs=2, space="PSUM"))
        psum_s = ctx.enter_context(tc.tile_pool(name="psum_s", bufs=1, space="PSUM"))

        # --- constants; interleave first enc group so PE starts ASAP ---
        covx_sb = singles.tile([2, BPC, T], bf16)
        nc.sync.dma_start(out=covx_sb[:], in_=covx_d[:])
        rhx_sb = singles.tile([2, BPC, H], bf16)
        nc.sync.dma_start(out=rhx_sb[:], in_=rhx_d[:])
        w2t_sb = singles.tile([128, KC, H], bf16)
        first_group = []
        for kc in range(KC):
            nc.sync.dma_start(out=w2t_sb[:, kc, :], in_=w2t_d[kc * 128:(kc + 1) * 128, :])
            et = encp.tile([128, 512], bf16, tag="enc_t0")
            nc.sync.dma_start(out=et[:], in_=enc_d[0, kc * 128:(kc + 1) * 128, 0:512])
            first_group.append(et)
        # rest of b0 rows in one medium DMA per k-chunk
        b0_rest = []
        for kc in range(KC):
            et = encp.tile([128, 3 * 512], bf16, tag="enc_t1")
            nc.sync.dma_start(out=et[:], in_=enc_d[0, kc * 128:(kc + 1) * 128, 512:T])
            b0_rest.append(et)
        # cold constants (not needed until the first epilogue) on another queue
        covt_sb = singles.tile([TC, BPC, 128], f32)
        nc.scalar.dma_start(
            out=covt_sb[:], in_=covt_d.rearrange("b q j -> q b j")
        )
        idn_sb = singles.tile([128, 128], f32)
        nc.scalar.dma_start(out=idn_sb[:], in_=idn_d[:])
        ones_c = singles.tile([TC, 1], f32)
        nc.vector.memset(ones_c[:], 1.0)
        ones_r = singles.tile([1, TC], f32)
        nc.vector.memset(ones_r[:], 1.0)

        big_tiles: dict = {}
        from contextlib import nullcontext
        rep_ctx = tc.For_i(0, reps, name="reploop") if reps > 1 else nullcontext()
        with rep_ctx:
          for b in range(BPC):
            sp = scorep.tile([128, TC], f32, tag="sp")
            sm = scorep.tile([128, TC], f32, tag="sm")
            if p_pos == 0:
                nc.vector.memset(sp[:], 0.0)
            if p_pos == H:
                nc.vector.memset(sm[:], 0.0)
            # prefetch the whole next batch as 4 big DMAs (fixed per-DMA
            # HWDGE descriptor-gen cost dominates; batch to amortize)
            if b + 1 < BPC:
                nxt = []
                for kc in range(KC):
                    et = encpb.tile([128, T], bf16, tag="enc_big")
                    nc.sync.dma_start(
                        out=et[:], in_=enc_d[b + 1, kc * 128:(kc + 1) * 128, :]
                    )
                    nxt.append(et)
                big_tiles[b + 1] = nxt
            for tcg in range(TCG):
                for j in range(4):
                    if mode == "dma":
                        continue
                    tci = tcg * 4 + j
                    pz = psum.tile([128, H], f32, tag="pz")
                    # K=2 rank-1 seed: [cov;1].T @ [u_s; a_b]  ->  cov*u + cb
                    nc.tensor.matmul(
                        pz[:],
                        lhsT=covx_sb[:, b, tci * 128:(tci + 1) * 128],
                        rhs=rhx_sb[:, b, :],
                        start=True,
                        stop=False,
                    )
                    for kc in range(KC):
                        if b == 0 and tcg == 0:
                            lhsT = first_group[kc][:, j * 128:(j + 1) * 128]
                        elif b == 0:
                            lhsT = b0_rest[kc][
                                :, (tcg - 1) * 512 + j * 128:(tcg - 1) * 512 + (j + 1) * 128
                            ]
                        else:
                            lhsT = big_tiles[b][kc][
                                :, tcg * 512 + j * 128:tcg * 512 + (j + 1) * 128
                            ]
                        nc.tensor.matmul(
                            pz[:],
                            lhsT=lhsT,
                            rhs=w2t_sb[:, kc, :],
                            start=False,
                            stop=(kc == KC - 1),
                        )
                    if mode == "mm":
                        continue
                    # relu + reduce in one DVE op per sign-slice:
                    # out = max(pz, 0); accum_out = reduce(out, op1=add)
                    if p_pos > 0:
                        scr = scrapp.tile([128, H], f32, tag="scr")
                        nc.vector.tensor_scalar(
                            out=scr[:, 0:p_pos],
                            in0=pz[:, 0:p_pos],
                            scalar1=0.0,
                            scalar2=None,
                            op0=Alu.max,
                            op1=Alu.add,
                            accum_out=sp[:, tci:tci + 1],
                        )
                    if p_pos < H:
                        scr2 = scrapp.tile([128, H], f32, tag="scr2")
                        nc.vector.tensor_scalar(
                            out=scr2[:, p_pos:H],
                            in0=pz[:, p_pos:H],
                            scalar1=0.0,
                            scalar2=None,
                            op0=Alu.max,
                            op1=Alu.add,
                            accum_out=sm[:, tci:tci + 1],
                        )

            # ---- per-batch epilogue: softmax over all 2048 t ----
            if mode in ("dma", "mm"):
                o = outp.tile([TC, 256], f32, tag="o")
                nc.vector.memset(o[:], 0.0)
                nc.sync.dma_start(out=out_d[b], in_=o[:])
                continue
            s_sb = smallp.tile([128, TC], f32, tag="s_sb")
            nc.vector.tensor_sub(s_sb[:], sp[:], sm[:])
            ps_t = psum_t.tile([TC, 128], f32, tag="ps_t")
            nc.tensor.transpose(ps_t[:], s_sb[:], idn_sb[:])
            # scores are O(3), so exp needs no max-subtraction (softmax is
            # shift-invariant; reference only subtracts max for range safety)
            expT = smallp.tile([TC, 128], f32, tag="expT")
            rsum = smallp.tile([TC, 1], f32, tag="rsum")
            nc.scalar.activation(
                out=expT[:], in_=ps_t[:], func=Act.Exp, accum_out=rsum[:]
            )
            # partition-sum and broadcast via tiny PE matmuls (gpsimd ops
            # force multi-ms ucode library reloads -- never use them here)
            zt_ps = psum_s.tile([1, 1], f32, tag="zt_ps")
            nc.tensor.matmul(zt_ps[:], lhsT=rsum[:], rhs=ones_c[:])
            zt = smallp.tile([1, 1], f32, tag="zt")
            nc.vector.tensor_copy(zt[:], zt_ps[:])
            rz = smallp.tile([1, 1], f32, tag="rz")
            nc.vector.reciprocal(rz[:], zt[:])
            rzb_ps = psum_s.tile([TC, 1], f32, tag="rzb_ps")
            nc.tensor.matmul(rzb_ps[:], lhsT=ones_r[:], rhs=rz[:])
            rzb = smallp.tile([TC, 1], f32, tag="rzb")
            nc.vector.tensor_copy(rzb[:], rzb_ps[:])
            o = outp.tile([TC, 256], f32, tag="o")
            nc.vector.tensor_scalar_mul(o[:, 0:128], expT[:], rzb[:])
            nc.vector.tensor_add(o[:, 128:256], o[:, 0:128], covt_sb[:, b, :])
            nc.sync.dma_start(out=out_d[b], in_=o[:])

    nc.compile()
    return nc


def _build_program_loop(reps: int = 1):
    """Looped (design C) program: weight-stationary matmuls inside a
    For_i hardware loop, v-reduction via PE matmul, batched softmax.

    The execution path charges ~35us per STATIC instruction (program
    upload), while looped execution runs at hardware speed -- so the
    whole kernel is structured as a compact 2-logical-iteration loop
    body (~110 static instructions) over 16 (batch, t-block) tiles.
    """
    from contextlib import ExitStack

    import concourse.tile as tile
    from concourse import bacc, mybir
    from concourse.bass import ds

    f32 = mybir.dt.float32
    bf16 = mybir.dt.bfloat16
    Alu = mybir.AluOpType
    Act = mybir.ActivationFunctionType

    NL = BPC * TCG                       # 16 logical tiles of [512k x 512t]

    nc = bacc.Bacc(
        "TRN2",
        target_bir_lowering=False,
        debug=False,
        enable_asserts=False,
        num_devices=N_CORES,
    )

    enc3_d = nc.dram_tensor("enc3_in", [NL, 128, KC, 512], bf16, kind="ExternalInput").ap()
    w2t_d = nc.dram_tensor("w2t_in", [128, KC, H], bf16, kind="ExternalInput").ap()
    covb_d = nc.dram_tensor("covb_in", [NL, 512], f32, kind="ExternalInput").ap()
    cb_d = nc.dram_tensor("cb_in", [NL, 128, KC], f32, kind="ExternalInput").ap()
    u_d = nc.dram_tensor("u_in", [128, KC], f32, kind="ExternalInput").ap()
    v_d = nc.dram_tensor("v_in", [128, KC], bf16, kind="ExternalInput").ap()
    em_d = nc.dram_tensor("em_in", [NL, BPC], f32, kind="ExternalInput").ap()
    emt_d = nc.dram_tensor("emt_in", [BPC, NL], f32, kind="ExternalInput").ap()
    out_d = nc.dram_tensor("out2_out", [NL, 1024], f32, kind="ExternalOutput").ap()

    with tile.TileContext(nc) as tc, ExitStack() as ctx:
        sg = ctx.enter_context(tc.tile_pool(name="sg", bufs=1))
        dramp = ctx.enter_context(tc.tile_pool(name="dramp", bufs=1, space="DRAM"))
        psp = ctx.enter_context(tc.tile_pool(name="psp", bufs=1, space="PSUM"))

        w2t_sb = sg.tile([128, KC, H], bf16)
        nc.sync.dma_start(out=w2t_sb[:], in_=w2t_d[:])
        u_sb = sg.tile([128, KC], f32)
        nc.sync.dma_start(out=u_sb[:], in_=u_d[:])
        v_sb = sg.tile([128, KC], bf16)
        nc.sync.dma_start(out=v_sb[:], in_=v_d[:])
        em_sb = sg.tile([NL, BPC], f32)
        nc.sync.dma_start(out=em_sb[:], in_=em_d[:])
        emt_sb = sg.tile([BPC, NL], f32)
        nc.sync.dma_start(out=emt_sb[:], in_=emt_d[:])
        covfull_sb = sg.tile([NL, 512], f32)
        nc.sync.dma_start(out=covfull_sb[:], in_=covb_d[:])

        UNROLL = 8
        sc_shared = [
            psp.tile([1, 512], f32, name=f"sc{j}", tag=f"sc{j}") for j in range(2)
        ]
        phases = []
        for pi, ph in enumerate(("A", "B", "C", "D", "E", "F", "G", "Hh")[:UNROLL]):
            phases.append(dict(
                enc=sg.tile([128, KC, 512], bf16, name=f"enc{ph}", tag=f"enc{ph}"),
                covb=sg.tile([128, 512], f32, name=f"covb{ph}", tag=f"covb{ph}"),
                cb=sg.tile([128, KC], f32, name=f"cb{ph}", tag=f"cb{ph}"),
                y=sg.tile([128, KC, 512], bf16, name=f"y{ph}", tag=f"y{ph}"),
                st=sg.tile([1, 512], f32, name=f"st{ph}", tag=f"st{ph}"),
                sc_ps=sc_shared[pi % 2],
            ))
        zt_ps = psp.tile([128, KC, H], f32)          # 4 PSUM banks
        scratch = dramp.tile([NL, 512], f32)

        from contextlib import nullcontext
        rep_ctx = tc.For_i(0, reps, name="reploop") if reps > 1 else nullcontext()
        with rep_ctx:
            with tc.For_i(0, NL // UNROLL, 1, staggered_reset=True) as i:
                # stage 0: all loads (next iteration's stage 0 may overlap
                # this iteration's stages 2-3 under staggered_reset)
                for phase, P in enumerate(phases):
                    l = i * UNROLL + phase
                    nc.sync.dma_start(out=P["enc"][:], in_=enc3_d[ds(l, 1), :, :, :])
                    nc.sync.dma_start(
                        out=P["covb"][:],
                        in_=covb_d[ds(l, 1), :][0].partition_broadcast(128),
                    )
                    nc.sync.dma_start(out=P["cb"][:], in_=cb_d[ds(l, 1), :, :])

                def main_compute(P):
                    for hc in range(KC):
                        for kc in range(KC):
                            nc.tensor.matmul(
                                zt_ps[:, hc, :],
                                lhsT=w2t_sb[:, kc, hc * 128:(hc + 1) * 128],
                                rhs=P["enc"][:, kc, :],
                                start=(kc == 0),
                                stop=(kc == KC - 1),
                            )
                        # z += cov[t]*u[h]  (in-place on PSUM)
                        nc.vector.scalar_tensor_tensor(
                            out=zt_ps[:, hc, :],
                            in0=P["covb"][:],
                            scalar=u_sb[:, hc:hc + 1],
                            in1=zt_ps[:, hc, :],
                            op0=Alu.mult,
                            op1=Alu.add,
                        )
                        # y = relu(z + a_b)  (bias is per-partition = per-h)
                        nc.scalar.activation(
                            out=P["y"][:, hc, :],
                            in_=zt_ps[:, hc, :],
                            func=Act.Relu,
                            bias=P["cb"][:, hc:hc + 1],
                        )

                def reduce_compute(P, l):
                    # scores[t] = v . y[:,t]  (contraction over h on PE)
                    for hc in range(KC):
                        nc.tensor.matmul(
                            P["sc_ps"][:],
                            lhsT=v_sb[:, hc:hc + 1],
                            rhs=P["y"][:, hc, :],
                            start=(hc == 0),
                            stop=(hc == KC - 1),
                        )
                    nc.vector.tensor_copy(P["st"][:], P["sc_ps"][:])
                    nc.sync.dma_start(out=scratch[ds(l, 1), :], in_=P["st"][:])

                for phase, P in enumerate(phases):
                    main_compute(P)
                    reduce_compute(P, i * UNROLL + phase)

            # ---- batched softmax epilogue over all 16 score rows ----
            sc16 = sg.tile([NL, 512], f32, tag="sc16")
            nc.sync.dma_start(out=sc16[:], in_=scratch[:])
            ex16 = sg.tile([NL, 512], f32, tag="ex16")
            rsum = sg.tile([NL, 1], f32, tag="rsum")
            nc.scalar.activation(out=ex16[:], in_=sc16[:], func=Act.Exp, accum_out=rsum[:])
            zb_ps = psp.tile([BPC, 1], f32, tag="zb_ps")
            nc.tensor.matmul(zb_ps[:], lhsT=em_sb[:], rhs=rsum[:])
            zb = sg.tile([BPC, 1], f32, tag="zb")
            nc.vector.tensor_copy(zb[:], zb_ps[:])
            rz = sg.tile([BPC, 1], f32, tag="rz")
            nc.vector.reciprocal(rz[:], zb[:])
            rzb_ps = psp.tile([NL, 1], f32, tag="rzb_ps")
            nc.tensor.matmul(rzb_ps[:], lhsT=emt_sb[:], rhs=rz[:])
            rzb = sg.tile([NL, 1], f32, tag="rzb")
            nc.vector.tensor_copy(rzb[:], rzb_ps[:])
            o16 = sg.tile([NL, 1024], f32, tag="o16")
            nc.vector.tensor_scalar_mul(o16[:, 0:512], ex16[:], rzb[:])
            nc.vector.tensor_add(o16[:, 512:1024], o16[:, 0:512], covfull_sb[:])
            nc.sync.dma_start(out=out_d[:], in_=o16[:])

    nc.compile()
    return nc


def _prepare_loop(hidden, encoder_outputs, coverage, W_attn, b_attn, v, W_cov):
    """Host-side sharding for the looped (design C) program."""
    hidden = np.asarray(hidden, dtype=np.float32)
    encoder_outputs = np.asarray(encoder_outputs, dtype=np.float32)
    coverage = np.asarray(coverage, dtype=np.float32)
    W_attn = np.asarray(W_attn, dtype=np.float32)
    b_attn = np.asarray(b_attn, dtype=np.float32)
    v = np.asarray(v, dtype=np.float32)
    W_cov = np.asarray(W_cov, dtype=np.float32)

    NL = BPC * TCG
    W1 = W_attn[:, :H].astype(np.float64)
    W2 = W_attn[:, H:2 * H].astype(np.float64)
    W3 = W_attn[:, 2 * H:].astype(np.float64)
    u = W3 @ W_cov[:, 0].astype(np.float64)                      # [H]
    a = hidden[0].astype(np.float64) @ W1.T + b_attn.astype(np.float64)  # [B,H]

    # [k, h] -> [p, kc, h]
    w2t = np.ascontiguousarray(
        W2.T.reshape(KC, 128, H).transpose(1, 0, 2)
    ).astype(np.float32).astype(ml_dtypes.bfloat16)
    u2 = np.ascontiguousarray(u.reshape(KC, 128).T).astype(np.float32)   # [p, hc]
    v2 = np.ascontiguousarray(
        v.reshape(KC, 128).T
    ).astype(ml_dtypes.bfloat16)                                          # [p, hc]

    em = np.zeros((NL, BPC), np.float32)
    for q in range(NL):
        em[q, q // TCG] = 1.0
    emt = np.ascontiguousarray(em.T)

    in_maps = []
    for c in range(N_CORES):
        bs = slice(c * BPC, (c + 1) * BPC)
        e2 = encoder_outputs[:, bs, :].transpose(1, 2, 0)        # [BPC, H, T]
        # [b, (kc p) k, (tb t') t] -> [l=(b tb), p, kc, t']
        enc3 = np.ascontiguousarray(
            e2.reshape(BPC, KC, 128, TCG, 512).transpose(0, 3, 2, 1, 4)
        ).reshape(NL, 128, KC, 512).astype(ml_dtypes.bfloat16)
        covb = np.ascontiguousarray(
            coverage[bs].reshape(NL, 512)
        ).astype(np.float32)
        ab = a[bs]                                               # [BPC, H]
        cb3 = np.empty((NL, 128, KC), np.float32)
        for b in range(BPC):
            blk = np.ascontiguousarray(ab[b].reshape(KC, 128).T).astype(np.float32)
            for tb in range(TCG):
                cb3[b * TCG + tb] = blk
        in_maps.append(
            {
                "enc3_in": enc3,
                "w2t_in": w2t,
                "covb_in": covb,
                "cb_in": cb3,
                "u_in": u2,
                "v_in": v2,
                "em_in": em,
                "emt_in": emt,
            }
        )
    return in_maps


def _get_program_loop(reps: int = 1):
    key = ("loop", reps)
    if key not in _PROGRAM_CACHE:
        _PROGRAM_CACHE[key] = _build_program_loop(reps)
    return _PROGRAM_CACHE[key]


def _run_loop(inputs: dict, trace: bool = False, reps: int = 1):
    from concourse import bass_utils

    in_maps = _prepare_loop(**inputs)
    nc = _get_program_loop(reps)
    res = bass_utils.run_bass_kernel_spmd(
        nc, in_maps, core_ids=list(range(N_CORES)), trace=trace
    )
    # out row l=(b,tb): [0:512]=attn block, [512:1024]=covn block
    outs = np.stack(
        [res.results[c]["out2_out"] for c in range(N_CORES)], axis=0
    ).astype(np.float32)                                         # [NC, NL, 1024]
    attn = np.ascontiguousarray(outs[:, :, 0:512]).reshape(N_CORES * BPC, T)
    covn = np.ascontiguousarray(outs[:, :, 512:1024]).reshape(N_CORES * BPC, T)
    return (attn[:, None, :], covn), res


def _get_program(p_pos: int, reps: int = 1, mode: str = "full"):
    key = (p_pos, reps, mode)
    if key not in _PROGRAM_CACHE:
        _PROGRAM_CACHE[key] = _build_program(p_pos, reps, mode)
    return _PROGRAM_CACHE[key]


def _prepare(hidden, encoder_outputs, coverage, W_attn, b_attn, v, W_cov):
    """Host-side sharding + weight folding. Returns (p_pos, in_maps)."""
    hidden = np.asarray(hidden, dtype=np.float32)
    encoder_outputs = np.asarray(encoder_outputs, dtype=np.float32)
    coverage = np.asarray(coverage, dtype=np.float32)
    W_attn = np.asarray(W_attn, dtype=np.float32)
    b_attn = np.asarray(b_attn, dtype=np.float32)
    v = np.asarray(v, dtype=np.float32)
    W_cov = np.asarray(W_cov, dtype=np.float32)

    W1 = W_attn[:, :H].astype(np.float64)
    W2 = W_attn[:, H:2 * H].astype(np.float64)
    W3 = W_attn[:, 2 * H:].astype(np.float64)
    u = W3 @ W_cov[:, 0].astype(np.float64)                      # [H]
    a = hidden[0].astype(np.float64) @ W1.T + b_attn.astype(np.float64)  # [B,H]

    order = np.argsort(v < 0, kind="stable")                     # v>=0 first
    p_pos = int((v >= 0).sum())
    vabs = np.abs(v[order].astype(np.float64))

    w2t_s = (W2[order, :].T * vabs[None, :])                     # [k, h']
    w2t_bf = w2t_s.astype(np.float32).astype(ml_dtypes.bfloat16)
    u_s = (u[order] * vabs).astype(np.float32)                   # [H]
    cb_s = (a[:, order] * vabs[None, :]).astype(np.float32)      # [B, H]

    ident = np.eye(128, dtype=np.float32)

    in_maps = []
    for c in range(N_CORES):
        bs = slice(c * BPC, (c + 1) * BPC)
        e = encoder_outputs[:, bs, :]                            # [T, BPC, H]
        enc_bf = np.ascontiguousarray(e.transpose(1, 2, 0)).astype(
            ml_dtypes.bfloat16
        )                                                        # [BPC, H, T]
        cov_c = coverage[bs]                                     # [BPC, T]
        covt = np.ascontiguousarray(cov_c.reshape(BPC, TC, 128))
        covx = np.empty((2, BPC, T), dtype=ml_dtypes.bfloat16)
        covx[0] = cov_c.astype(ml_dtypes.bfloat16)
        covx[1] = np.float32(1.0)
        rhx = np.empty((2, BPC, H), dtype=ml_dtypes.bfloat16)
        rhx[0] = u_s[None, :].astype(ml_dtypes.bfloat16)
        rhx[1] = cb_s[bs].astype(ml_dtypes.bfloat16)
        in_maps.append(
            {
                "enc_in": enc_bf,
                "w2t_in": w2t_bf,
                "covx_in": covx,
                "rhx_in": rhx,
                "covt_in": covt,
                "iden_in": ident,
            }
        )
    return p_pos, in_maps


def _run(inputs: dict, trace: bool = False, reps: int = 1, mode: str = "full"):
    """Run on 8 NeuronCores. Returns ((attn, covnew), BassKernelResults)."""
    from concourse import bass_utils

    p_pos, in_maps = _prepare(**inputs)
    nc = _get_program(p_pos, reps, mode)
    res = bass_utils.run_bass_kernel_spmd(
        nc, in_maps, core_ids=list(range(N_CORES)), trace=trace
    )
    outs = np.concatenate(
        [res.results[c]["out2_out"] for c in range(N_CORES)], axis=0
    ).astype(np.float32)                                        # [B, TC, 256]
    attn = np.ascontiguousarray(outs[:, :, 0:128]).reshape(B, T)
    covn = np.ascontiguousarray(outs[:, :, 128:256]).reshape(B, T)
    return (attn[:, None, :], covn), res


def kernel(hidden, encoder_outputs, coverage, W_attn, b_attn, v, W_cov):
    out, _ = _run_loop(
        dict(
            hidden=hidden,
            encoder_outputs=encoder_outputs,
            coverage=coverage,
            W_attn=W_attn,
            b_attn=b_attn,
            v=v,
            W_cov=W_cov,
        )
    )
    return out


# revision 47
# speedup vs baseline: 441.2150x; 2.1279x over previous
"""Trainium2 Bass kernel for coverage-attention (Bahdanau + coverage).

Reference computation (fp32):
    enc   = encoder_outputs.transpose(1,0,2)            # [B,T,H]
    x     = concat([hidden_rep, enc, cov*W_cov], -1)    # [B,T,3H]
    energy= relu(x @ W_attn.T + b_attn)                 # [B,T,H]
    scores= energy @ v                                  # [B,T]
    attn  = softmax(scores, axis=1)
    out   = (attn[:,None,:], coverage + attn)

Decomposition (W_attn = [W1 | W2 | W3], each [H,H]):
    z[b,t,h] = (enc @ W2.T)[b,t,h] + a[b,h] + cov[b,t]*u[h]
      a = hidden[0] @ W1.T + b_attn      (tiny, host-precomputed)
      u = W3 @ W_cov[:,0]                (tiny, host-precomputed)
    scores[b,t] = sum_h v[h]*relu(z[b,t,h])
Only enc @ W2.T is large; everything else is a rank-1/bias correction.

Sharding: data-parallel over batch, 4 batches per core x 8 cores.

The shipped implementation (_build_program_loop, "design C") is a
weight-stationary kernel wrapped in a For_i hardware loop:
  - per (batch, 512-t-block) tile, layout [128h x 512t]:
      PE:  4x4 bf16 matmuls  w2t[kc,hc].T @ enc[kc]  -> PSUM z
      DVE: scalar_tensor_tensor  z += cov[t]*u[h]    (one op, in-place)
      ACT: y = relu(z + a_b)   (bias rides the activation, out bf16)
      PE:  score[1,512t] = v.T @ y   (4 accumulating matmuls)
  - scores bounce through a DRAM scratch row per tile; one batched
    softmax epilogue (exp+accum on ACT, per-batch sums/broadcast via
    tiny indicator-matrix matmuls on PE, no max-subtraction needed
    since |score| < ~3 and softmax is shift-invariant).
  - loop: For_i over 2 iterations x 8-tile unrolled body with
    staggered_reset (cheap back-edge, cross-iteration DMA overlap).

Why the loop: on this (axon-tunneled) stack each STATIC instruction
costs ~35us per execution (program upload), while looped execution
runs at silicon speed; the loop body keeps the program ~600
instructions.  gpsimd is avoided entirely (its ops trigger multi-ms
ucode library reloads).  Measured ~215-270us on-device per invocation
across 8 cores, rel err ~1.7e-3 (bf16 matmul datapath, fp32 softmax).

An alternative fully-unrolled implementation (design A: t-on-partition
tiles, relu+reduce via DVE tensor_scalar accumulate) is kept as
_build_program/_run for reference; it measures ~265us device time.
"""

import os
import sys

import numpy as np

for _p in ("/opt/trn_rl_repo", "/root/.axon_site/_ro/trn_rl_repo"):
    if os.path.isdir(_p) and _p not in sys.path:
        sys.path.insert(0, _p)
        break

import ml_dtypes  # noqa: E402

H = 512
B = 32
T = 2048
N_CORES = 8
BPC = B // N_CORES          # batches per core
TC = T // 128               # 16 score columns (t-tiles) per batch
TCG = 4                     # t-tile groups of 512 t each
KC = 4                      # k chunks of 128

_PROGRAM_CACHE: dict = {}


def _build_program(p_pos: int, reps: int = 1, mode: str = "full"):
    """Build + compile the single-core Bass/Tile program (SPMD across 8).

    reps>1 repeats the whole computation back-to-back inside one NEFF
    (idempotent), for wall-clock benchmarking that cancels host overhead.
    mode: "full" | "dma" (loads only) | "mm" (loads+matmuls) for bisection.
    """
    from contextlib import ExitStack

    import concourse.tile as tile
    from concourse import bacc, mybir

    f32 = mybir.dt.float32
    bf16 = mybir.dt.bfloat16
    Alu = mybir.AluOpType
    Act = mybir.ActivationFunctionType

    nc = bacc.Bacc(
        "TRN2",
        target_bir_lowering=False,
        debug=False,
        enable_asserts=False,
        num_devices=N_CORES,
    )

    enc_d = nc.dram_tensor("enc_in", [BPC, H, T], bf16, kind="ExternalInput").ap()
    w2t_d = nc.dram_tensor("w2t_in", [H, H], bf16, kind="ExternalInput").ap()
    covx_d = nc.dram_tensor("covx_in", [2, BPC, T], bf16, kind="ExternalInput").ap()
    rhx_d = nc.dram_tensor("rhx_in", [2, BPC, H], bf16, kind="ExternalInput").ap()
    covt_d = nc.dram_tensor("covt_in", [BPC, TC, 128], f32, kind="ExternalInput").ap()
    idn_d = nc.dram_tensor("iden_in", [128, 128], f32, kind="ExternalInput").ap()
    # fused output: [..., 0:128] = attn, [..., 128:256] = coverage_new
    out_d = nc.dram_tensor("out2_out", [BPC, TC, 256], f32, kind="ExternalOutput").ap()

    with tile.TileContext(nc) as tc, ExitStack() as ctx:
        singles = ctx.enter_context(tc.tile_pool(name="singles", bufs=1))
        encp = ctx.enter_context(tc.tile_pool(name="encp", bufs=4))
        encpb = ctx.enter_context(tc.tile_pool(name="encpb", bufs=8))
        scrapp = ctx.enter_context(tc.tile_pool(name="scrapp", bufs=3))
        scorep = ctx.enter_context(tc.tile_pool(name="scorep", bufs=2))
        smallp = ctx.enter_context(tc.tile_pool(name="smallp", bufs=2))
        outp = ctx.enter_context(tc.tile_pool(name="outp", bufs=2))
        psum = ctx.enter_context(tc.tile_pool(name="psum", bufs=4, space="PSUM"))
        psum_t = ctx.enter_context(tc.tile_pool(name="psum_t", bufs=2, space="PSUM"))
        psum_s = ctx.enter_context(tc.tile_pool(name="psum_s", bufs=1, space="PSUM"))

        # --- constants; interleave first enc group so PE starts ASAP ---
        covx_sb = singles.tile([2, BPC, T], bf16)
        nc.sync.dma_start(out=covx_sb[:], in_=covx_d[:])
        rhx_sb = singles.tile([2, BPC, H], bf16)
        nc.sync.dma_start(out=rhx_sb[:], in_=rhx_d[:])
        w2t_sb = singles.tile([128, KC, H], bf16)
        first_group = []
        for kc in range(KC):
            nc.sync.dma_start(out=w2t_sb[:, kc, :], in_=w2t_d[kc * 128:(kc + 1) * 128, :])
            et = encp.tile([128, 512], bf16, tag="enc_t0")
            nc.sync.dma_start(out=et[:], in_=enc_d[0, kc * 128:(kc + 1) * 128, 0:512])
            first_group.append(et)
        # rest of b0 rows in one medium DMA per k-chunk
        b0_rest = []
        for kc in range(KC):
            et = encp.tile([128, 3 * 512], bf16, tag="enc_t1")
            nc.sync.dma_start(out=et[:], in_=enc_d[0, kc * 128:(kc + 1) * 128, 512:T])
            b0_rest.append(et)
        # cold constants (not needed until the first epilogue) on another queue
        covt_sb = singles.tile([TC, BPC, 128], f32)
        nc.scalar.dma_start(
            out=covt_sb[:], in_=covt_d.rearrange("b q j -> q b j")
        )
        idn_sb = singles.tile([128, 128], f32)
        nc.scalar.dma_start(out=idn_sb[:], in_=idn_d[:])
        ones_c = singles.tile([TC, 1], f32)
        nc.vector.memset(ones_c[:], 1.0)
        ones_r = singles.tile([1, TC], f32)
        nc.vector.memset(ones_r[:], 1.0)

        big_tiles: dict = {}
        from contextlib import nullcontext
        rep_ctx = tc.For_i(0, reps, name="reploop") if reps > 1 else nullcontext()
        with rep_ctx:
          for b in range(BPC):
            sp = scorep.tile([128, TC], f32, tag="sp")
            sm = scorep.tile([128, TC], f32, tag="sm")
            if p_pos == 0:
                nc.vector.memset(sp[:], 0.0)
            if p_pos == H:
                nc.vector.memset(sm[:], 0.0)
            # prefetch the whole next batch as 4 big DMAs (fixed per-DMA
            # HWDGE descriptor-gen cost dominates; batch to amortize)
            if b + 1 < BPC:
                nxt = []
                for kc in range(KC):
                    et = encpb.tile([128, T], bf16, tag="enc_big")
                    nc.sync.dma_start(
                        out=et[:], in_=enc_d[b + 1, kc * 128:(kc + 1) * 128, :]
                    )
                    nxt.append(et)
                big_tiles[b + 1] = nxt
            for tcg in range(TCG):
                for j in range(4):
                    if mode == "dma":
                        continue
                    tci = tcg * 4 + j
                    pz = psum.tile([128, H], f32, tag="pz")
                    # K=2 rank-1 seed: [cov;1].T @ [u_s; a_b]  ->  cov*u + cb
                    nc.tensor.matmul(
                        pz[:],
                        lhsT=covx_sb[:, b, tci * 128:(tci + 1) * 128],
                        rhs=rhx_sb[:, b, :],
                        start=True,
                        stop=False,
                    )
                    for kc in range(KC):
                        if b == 0 and tcg == 0:
                            lhsT = first_group[kc][:, j * 128:(j + 1) * 128]
                        elif b == 0:
                            lhsT = b0_rest[kc][
                                :, (tcg - 1) * 512 + j * 128:(tcg - 1) * 512 + (j + 1) * 128
                            ]
                        else:
                            lhsT = big_tiles[b][kc][
                                :, tcg * 512 + j * 128:tcg * 512 + (j + 1) * 128
                            ]
                        nc.tensor.matmul(
                            pz[:],
                            lhsT=lhsT,
                            rhs=w2t_sb[:, kc, :],
                            start=False,
                            stop=(kc == KC - 1),
                        )
                    if mode == "mm":
                        continue
                    # relu + reduce in one DVE op per sign-slice:
                    # out = max(pz, 0); accum_out = reduce(out, op1=add)
                    if p_pos > 0:
                        scr = scrapp.tile([128, H], f32, tag="scr")
                        nc.vector.tensor_scalar(
                            out=scr[:, 0:p_pos],
                            in0=pz[:, 0:p_pos],
                            scalar1=0.0,
                            scalar2=None,
                            op0=Alu.max,
                            op1=Alu.add,
                            accum_out=sp[:, tci:tci + 1],
                        )
                    if p_pos < H:
                        scr2 = scrapp.tile([128, H], f32, tag="scr2")
                        nc.vector.tensor_scalar(
                            out=scr2[:, p_pos:H],
                            in0=pz[:, p_pos:H],
                            scalar1=0.0,
                            scalar2=None,
                            op0=Alu.max,
                            op1=Alu.add,
                            accum_out=sm[:, tci:tci + 1],
                        )

            # ---- per-batch epilogue: softmax over all 2048 t ----
            if mode in ("dma", "mm"):
                o = outp.tile([TC, 256], f32, tag="o")
                nc.vector.memset(o[:], 0.0)
                nc.sync.dma_start(out=out_d[b], in_=o[:])
                continue
            s_sb = smallp.tile([128, TC], f32, tag="s_sb")
            nc.vector.tensor_sub(s_sb[:], sp[:], sm[:])
            ps_t = psum_t.tile([TC, 128], f32, tag="ps_t")
            nc.tensor.transpose(ps_t[:], s_sb[:], idn_sb[:])
            # scores are O(3), so exp needs no max-subtraction (softmax is
            # shift-invariant; reference only subtracts max for range safety)
            expT = smallp.tile([TC, 128], f32, tag="expT")
            rsum = smallp.tile([TC, 1], f32, tag="rsum")
            nc.scalar.activation(
                out=expT[:], in_=ps_t[:], func=Act.Exp, accum_out=rsum[:]
            )
            # partition-sum and broadcast via tiny PE matmuls (gpsimd ops
            # force multi-ms ucode library reloads -- never use them here)
            zt_ps = psum_s.tile([1, 1], f32, tag="zt_ps")
            nc.tensor.matmul(zt_ps[:], lhsT=rsum[:], rhs=ones_c[:])
            zt = smallp.tile([1, 1], f32, tag="zt")
            nc.vector.tensor_copy(zt[:], zt_ps[:])
            rz = smallp.tile([1, 1], f32, tag="rz")
            nc.vector.reciprocal(rz[:], zt[:])
            rzb_ps = psum_s.tile([TC, 1], f32, tag="rzb_ps")
            nc.tensor.matmul(rzb_ps[:], lhsT=ones_r[:], rhs=rz[:])
            rzb = smallp.tile([TC, 1], f32, tag="rzb")
            nc.vector.tensor_copy(rzb[:], rzb_ps[:])
            o = outp.tile([TC, 256], f32, tag="o")
            nc.vector.tensor_scalar_mul(o[:, 0:128], expT[:], rzb[:])
            nc.vector.tensor_add(o[:, 128:256], o[:, 0:128], covt_sb[:, b, :])
            nc.sync.dma_start(out=out_d[b], in_=o[:])

    nc.compile()
    return nc


def _build_program_loop(reps: int = 1):
    """Looped (design C) program: weight-stationary matmuls inside a
    For_i hardware loop, v-reduction via PE matmul, batched softmax.

    The execution path charges ~35us per STATIC instruction (program
    upload), while looped execution runs at hardware speed -- so the
    whole kernel is structured as a compact 2-logical-iteration loop
    body (~110 static instructions) over 16 (batch, t-block) tiles.
    """
    from contextlib import ExitStack

    import concourse.tile as tile
    from concourse import bacc, mybir
    from concourse.bass import ds

    f32 = mybir.dt.float32
    bf16 = mybir.dt.bfloat16
    Alu = mybir.AluOpType
    Act = mybir.ActivationFunctionType

    NL = BPC * TCG                       # 16 logical tiles of [512k x 512t]

    nc = bacc.Bacc(
        "TRN2",
        target_bir_lowering=False,
        debug=False,
        enable_asserts=False,
        num_devices=N_CORES,
    )

    enc3_d = nc.dram_tensor("enc3_in", [NL, 128, KC, 512], bf16, kind="ExternalInput").ap()
    w2t_d = nc.dram_tensor("w2t_in", [128, KC, H], bf16, kind="ExternalInput").ap()
    covb_d = nc.dram_tensor("covb_in", [NL, 512], f32, kind="ExternalInput").ap()
    cb_d = nc.dram_tensor("cb_in", [NL, 128, KC], f32, kind="ExternalInput").ap()
    u_d = nc.dram_tensor("u_in", [128, KC], f32, kind="ExternalInput").ap()
    v_d = nc.dram_tensor("v_in", [128, KC], bf16, kind="ExternalInput").ap()
    em_d = nc.dram_tensor("em_in", [NL, BPC], f32, kind="ExternalInput").ap()
    emt_d = nc.dram_tensor("emt_in", [BPC, NL], f32, kind="ExternalInput").ap()
    out_d = nc.dram_tensor("out2_out", [NL, 1024], f32, kind="ExternalOutput").ap()

    with tile.TileContext(nc) as tc, ExitStack() as ctx:
        sg = ctx.enter_context(tc.tile_pool(name="sg", bufs=1))
        dramp = ctx.enter_context(tc.tile_pool(name="dramp", bufs=1, space="DRAM"))
        psp = ctx.enter_context(tc.tile_pool(name="psp", bufs=1, space="PSUM"))

        w2t_sb = sg.tile([128, KC, H], bf16)
        nc.sync.dma_start(out=w2t_sb[:], in_=w2t_d[:])
        u_sb = sg.tile([128, KC], f32)
        nc.sync.dma_start(out=u_sb[:], in_=u_d[:])
        v_sb = sg.tile([128, KC], bf16)
        nc.sync.dma_start(out=v_sb[:], in_=v_d[:])
        em_sb = sg.tile([NL, BPC], f32)
        nc.sync.dma_start(out=em_sb[:], in_=em_d[:])
        emt_sb = sg.tile([BPC, NL], f32)
        nc.sync.dma_start(out=emt_sb[:], in_=emt_d[:])
        covfull_sb = sg.tile([NL, 512], f32)
        nc.sync.dma_start(out=covfull_sb[:], in_=covb_d[:])

        UNROLL = 8
        sc_shared = [
            psp.tile([1, 512], f32, name=f"sc{j}", tag=f"sc{j}") for j in range(2)
        ]
        phases = []
        for pi, ph in enumerate([f"P{j}" for j in range(UNROLL)]):
            phases.append(dict(
                enc=sg.tile([128, KC, 512], bf16, name=f"enc{ph}", tag=f"enc{ph}"),
                covb=sg.tile([128, 512], f32, name=f"covb{ph}", tag=f"covb{ph}"),
                cb=sg.tile([128, KC], f32, name=f"cb{ph}", tag=f"cb{ph}"),
                y=sg.tile([128, KC, 512], bf16, name=f"y{ph}", tag=f"y{ph}"),
                st=sg.tile([1, 512], f32, name=f"st{ph}", tag=f"st{ph}"),
                sc_ps=sc_shared[pi % 2],
            ))
        zt_ps = psp.tile([128, KC, H], f32)          # 4 PSUM banks
        scratch = dramp.tile([NL, 512], f32)

        from contextlib import nullcontext
        rep_ctx = tc.For_i(0, reps, name="reploop") if reps > 1 else nullcontext()
        with rep_ctx:
            with tc.For_i(0, NL // UNROLL, 1, staggered_reset=True) as i:
                # stage 0: all loads (next iteration's stage 0 may overlap
                # this iteration's stages 2-3 under staggered_reset)
                for phase, P in enumerate(phases):
                    l = i * UNROLL + phase
                    nc.sync.dma_start(out=P["enc"][:], in_=enc3_d[ds(l, 1), :, :, :])
                    nc.sync.dma_start(
                        out=P["covb"][:],
                        in_=covb_d[ds(l, 1), :][0].partition_broadcast(128),
                    )
                    nc.sync.dma_start(out=P["cb"][:], in_=cb_d[ds(l, 1), :, :])

                def main_compute(P):
                    for hc in range(KC):
                        for kc in range(KC):
                            nc.tensor.matmul(
                                zt_ps[:, hc, :],
                                lhsT=w2t_sb[:, kc, hc * 128:(hc + 1) * 128],
                                rhs=P["enc"][:, kc, :],
                                start=(kc == 0),
                                stop=(kc == KC - 1),
                            )
                        # z += cov[t]*u[h]  (in-place on PSUM)
                        nc.vector.scalar_tensor_tensor(
                            out=zt_ps[:, hc, :],
                            in0=P["covb"][:],
                            scalar=u_sb[:, hc:hc + 1],
                            in1=zt_ps[:, hc, :],
                            op0=Alu.mult,
                            op1=Alu.add,
                        )
                        # y = relu(z + a_b)  (bias is per-partition = per-h)
                        nc.scalar.activation(
                            out=P["y"][:, hc, :],
                            in_=zt_ps[:, hc, :],
                            func=Act.Relu,
                            bias=P["cb"][:, hc:hc + 1],
                        )

                def reduce_compute(P, l):
                    # scores[t] = v . y[:,t]  (contraction over h on PE)
                    for hc in range(KC):
                        nc.tensor.matmul(
                            P["sc_ps"][:],
                            lhsT=v_sb[:, hc:hc + 1],
                            rhs=P["y"][:, hc, :],
                            start=(hc == 0),
                            stop=(hc == KC - 1),
                        )
                    nc.vector.tensor_copy(P["st"][:], P["sc_ps"][:])
                    nc.sync.dma_start(out=scratch[ds(l, 1), :], in_=P["st"][:])

                for phase, P in enumerate(phases):
                    main_compute(P)
                    reduce_compute(P, i * UNROLL + phase)

            # ---- batched softmax epilogue over all 16 score rows ----
            sc16 = sg.tile([NL, 512], f32, tag="sc16")
            nc.sync.dma_start(out=sc16[:], in_=scratch[:])
            ex16 = sg.tile([NL, 512], f32, tag="ex16")
            rsum = sg.tile([NL, 1], f32, tag="rsum")
            nc.scalar.activation(out=ex16[:], in_=sc16[:], func=Act.Exp, accum_out=rsum[:])
            zb_ps = psp.tile([BPC, 1], f32, tag="zb_ps")
            nc.tensor.matmul(zb_ps[:], lhsT=em_sb[:], rhs=rsum[:])
            zb = sg.tile([BPC, 1], f32, tag="zb")
            nc.vector.tensor_copy(zb[:], zb_ps[:])
            rz = sg.tile([BPC, 1], f32, tag="rz")
            nc.vector.reciprocal(rz[:], zb[:])
            rzb_ps = psp.tile([NL, 1], f32, tag="rzb_ps")
            nc.tensor.matmul(rzb_ps[:], lhsT=emt_sb[:], rhs=rz[:])
            rzb = sg.tile([NL, 1], f32, tag="rzb")
            nc.vector.tensor_copy(rzb[:], rzb_ps[:])
            o16 = sg.tile([NL, 1024], f32, tag="o16")
            nc.vector.tensor_scalar_mul(o16[:, 0:512], ex16[:], rzb[:])
            nc.vector.tensor_add(o16[:, 512:1024], o16[:, 0:512], covfull_sb[:])
            nc.sync.dma_start(out=out_d[:], in_=o16[:])

    nc.compile()
    return nc


def _prepare_loop(hidden, encoder_outputs, coverage, W_attn, b_attn, v, W_cov):
    """Host-side sharding for the looped (design C) program."""
    hidden = np.asarray(hidden, dtype=np.float32)
    encoder_outputs = np.asarray(encoder_outputs, dtype=np.float32)
    coverage = np.asarray(coverage, dtype=np.float32)
    W_attn = np.asarray(W_attn, dtype=np.float32)
    b_attn = np.asarray(b_attn, dtype=np.float32)
    v = np.asarray(v, dtype=np.float32)
    W_cov = np.asarray(W_cov, dtype=np.float32)

    NL = BPC * TCG
    W1 = W_attn[:, :H].astype(np.float64)
    W2 = W_attn[:, H:2 * H].astype(np.float64)
    W3 = W_attn[:, 2 * H:].astype(np.float64)
    u = W3 @ W_cov[:, 0].astype(np.float64)                      # [H]
    a = hidden[0].astype(np.float64) @ W1.T + b_attn.astype(np.float64)  # [B,H]

    # [k, h] -> [p, kc, h]
    w2t = np.ascontiguousarray(
        W2.T.reshape(KC, 128, H).transpose(1, 0, 2)
    ).astype(np.float32).astype(ml_dtypes.bfloat16)
    u2 = np.ascontiguousarray(u.reshape(KC, 128).T).astype(np.float32)   # [p, hc]
    v2 = np.ascontiguousarray(
        v.reshape(KC, 128).T
    ).astype(ml_dtypes.bfloat16)                                          # [p, hc]

    em = np.zeros((NL, BPC), np.float32)
    for q in range(NL):
        em[q, q // TCG] = 1.0
    emt = np.ascontiguousarray(em.T)

    in_maps = []
    for c in range(N_CORES):
        bs = slice(c * BPC, (c + 1) * BPC)
        e2 = encoder_outputs[:, bs, :].transpose(1, 2, 0)        # [BPC, H, T]
        # [b, (kc p) k, (tb t') t] -> [l=(b tb), p, kc, t']
        enc3 = np.ascontiguousarray(
            e2.reshape(BPC, KC, 128, TCG, 512).transpose(0, 3, 2, 1, 4)
        ).reshape(NL, 128, KC, 512).astype(ml_dtypes.bfloat16)
        covb = np.ascontiguousarray(
            coverage[bs].reshape(NL, 512)
        ).astype(np.float32)
        ab = a[bs]                                               # [BPC, H]
        cb3 = np.empty((NL, 128, KC), np.float32)
        for b in range(BPC):
            blk = np.ascontiguousarray(ab[b].reshape(KC, 128).T).astype(np.float32)
            for tb in range(TCG):
                cb3[b * TCG + tb] = blk
        in_maps.append(
            {
                "enc3_in": enc3,
                "w2t_in": w2t,
                "covb_in": covb,
                "cb_in": cb3,
                "u_in": u2,
                "v_in": v2,
                "em_in": em,
                "emt_in": emt,
            }
        )
    return in_maps


def _get_program_loop(reps: int = 1):
    key = ("loop", reps)
    if key not in _PROGRAM_CACHE:
        _PROGRAM_CACHE[key] = _build_program_loop(reps)
    return _PROGRAM_CACHE[key]


def _run_loop(inputs: dict, trace: bool = False, reps: int = 1):
    from concourse import bass_utils

    in_maps = _prepare_loop(**inputs)
    nc = _get_program_loop(reps)
    res = bass_utils.run_bass_kernel_spmd(
        nc, in_maps, core_ids=list(range(N_CORES)), trace=trace
    )
    # out row l=(b,tb): [0:512]=attn block, [512:1024]=covn block
    outs = np.stack(
        [res.results[c]["out2_out"] for c in range(N_CORES)], axis=0
    ).astype(np.float32)                                         # [NC, NL, 1024]
    attn = np.ascontiguousarray(outs[:, :, 0:512]).reshape(N_CORES * BPC, T)
    covn = np.ascontiguousarray(outs[:, :, 512:1024]).reshape(N_CORES * BPC, T)
    return (attn[:, None, :], covn), res


def _get_program(p_pos: int, reps: int = 1, mode: str = "full"):
    key = (p_pos, reps, mode)
    if key not in _PROGRAM_CACHE:
        _PROGRAM_CACHE[key] = _build_program(p_pos, reps, mode)
    return _PROGRAM_CACHE[key]


def _prepare(hidden, encoder_outputs, coverage, W_attn, b_attn, v, W_cov):
    """Host-side sharding + weight folding. Returns (p_pos, in_maps)."""
    hidden = np.asarray(hidden, dtype=np.float32)
    encoder_outputs = np.asarray(encoder_outputs, dtype=np.float32)
    coverage = np.asarray(coverage, dtype=np.float32)
    W_attn = np.asarray(W_attn, dtype=np.float32)
    b_attn = np.asarray(b_attn, dtype=np.float32)
    v = np.asarray(v, dtype=np.float32)
    W_cov = np.asarray(W_cov, dtype=np.float32)

    W1 = W_attn[:, :H].astype(np.float64)
    W2 = W_attn[:, H:2 * H].astype(np.float64)
    W3 = W_attn[:, 2 * H:].astype(np.float64)
    u = W3 @ W_cov[:, 0].astype(np.float64)                      # [H]
    a = hidden[0].astype(np.float64) @ W1.T + b_attn.astype(np.float64)  # [B,H]

    order = np.argsort(v < 0, kind="stable")                     # v>=0 first
    p_pos = int((v >= 0).sum())
    vabs = np.abs(v[order].astype(np.float64))

    w2t_s = (W2[order, :].T * vabs[None, :])                     # [k, h']
    w2t_bf = w2t_s.astype(np.float32).astype(ml_dtypes.bfloat16)
    u_s = (u[order] * vabs).astype(np.float32)                   # [H]
    cb_s = (a[:, order] * vabs[None, :]).astype(np.float32)      # [B, H]

    ident = np.eye(128, dtype=np.float32)

    in_maps = []
    for c in range(N_CORES):
        bs = slice(c * BPC, (c + 1) * BPC)
        e = encoder_outputs[:, bs, :]                            # [T, BPC, H]
        enc_bf = np.ascontiguousarray(e.transpose(1, 2, 0)).astype(
            ml_dtypes.bfloat16
        )                                                        # [BPC, H, T]
        cov_c = coverage[bs]                                     # [BPC, T]
        covt = np.ascontiguousarray(cov_c.reshape(BPC, TC, 128))
        covx = np.empty((2, BPC, T), dtype=ml_dtypes.bfloat16)
        covx[0] = cov_c.astype(ml_dtypes.bfloat16)
        covx[1] = np.float32(1.0)
        rhx = np.empty((2, BPC, H), dtype=ml_dtypes.bfloat16)
        rhx[0] = u_s[None, :].astype(ml_dtypes.bfloat16)
        rhx[1] = cb_s[bs].astype(ml_dtypes.bfloat16)
        in_maps.append(
            {
                "enc_in": enc_bf,
                "w2t_in": w2t_bf,
                "covx_in": covx,
                "rhx_in": rhx,
                "covt_in": covt,
                "iden_in": ident,
            }
        )
    return p_pos, in_maps


def _run(inputs: dict, trace: bool = False, reps: int = 1, mode: str = "full"):
    """Run on 8 NeuronCores. Returns ((attn, covnew), BassKernelResults)."""
    from concourse import bass_utils

    p_pos, in_maps = _prepare(**inputs)
    nc = _get_program(p_pos, reps, mode)
    res = bass_utils.run_bass_kernel_spmd(
        nc, in_maps, core_ids=list(range(N_CORES)), trace=trace
    )
    outs = np.concatenate(
        [res.results[c]["out2_out"] for c in range(N_CORES)], axis=0
    ).astype(np.float32)                                        # [B, TC, 256]
    attn = np.ascontiguousarray(outs[:, :, 0:128]).reshape(B, T)
    covn = np.ascontiguousarray(outs[:, :, 128:256]).reshape(B, T)
    return (attn[:, None, :], covn), res


def kernel(hidden, encoder_outputs, coverage, W_attn, b_attn, v, W_cov):
    out, _ = _run_loop(
        dict(
            hidden=hidden,
            encoder_outputs=encoder_outputs,
            coverage=coverage,
            W_attn=W_attn,
            b_attn=b_attn,
            v=v,
            W_cov=W_cov,
        )
    )
    return out
